# Initial kernel scaffold
#
"""Trainium2 Bass kernel for nn_DynamicSelectiveHyperNet.

Strategy
--------
Shard the target-parameter axis T across the 8 NeuronCores (no collectives;
the gated head-sum is computed locally per T-slice). Each core runs all 8
heads for its slice. The kernel is HBM-byte-bound, so the streamed weights
are compressed:

  att stream  [H, 68, ts]   fp8-e4m3, values x128:
      rows 0..63  = att_W[h, t, 0:64].T          (feats part)
      rows 64..67 = F[h, p, t] = embeds[p] . att_W[h, t, 64:96] + att_b[h, t]
      (host-precomputed, weights-only: the rank-4 embeds block + bias fold
       96+1 contraction rows down to 4)
  gen stream  [H, 128, ts/4] fp8-e4m3 x128: gen_W2 with the 4 column groups
      of each 2048-wide supertile folded into the contraction dim, so one
      K=128 matmul (vs 4) covers a supertile; bf16 block-diagonal stationary.
  gen bias    [H, 1, ts]    bf16 (fp8 bias fails the 2e-2 gate)

Per (head, supertile): 4x K=68 fp8 att matmuls (logits x2048, un-scaled by
the sigmoid's ACT scale), 1x K=128 mixed fp8/bf16 gen matmul, 4x K=1 bias
matmuls accumulating into the same PSUM; sigmoid on ACT; fused
scale-multiply + add on DVE. Output is written in bf16 (harness-side cast
back to fp32). Measured rel err (absmax-normalized): 1.33e-2 vs the 2e-2
gate; all-fp8 gen (2.1e-2+) and fp8 bias (2.7e-2) fail, bf16-gen (4.6e-3)
is the fallback via GEN_COMB=GEN_MIXED=False.

The preamble (feature extractor, gate softmax, per-head hmid, stationaries)
runs inside the repeat loop used for timing, so amortized per-iteration
numbers include it. DMA rings: att+consts+out on SP, gen on Activation
(gpsimd SWDGE DMAs crash this walrus build; bandwidth is shared anyway).
"""

import sys

sys.path.insert(0, "/opt/trn_rl_repo")

import json

import numpy as np

import concourse.bass as bass
import concourse.bass2jax as _bass2jax
import concourse.bass_utils as _bass_utils
import concourse.tile as tile
from concourse import mybir
from concourse.bass_utils import run_bass_kernel_spmd

AF = mybir.ActivationFunctionType
ALU = mybir.AluOpType
F32 = mybir.dt.float32
BF16 = mybir.dt.bfloat16
F8 = mybir.dt.float8e4
AX = mybir.AxisListType

B = 8
H = 8
NP = 4          # target param groups
FEAT = 64
EMB = 32
HIN = 96        # FEAT + EMB
GH = 32         # generator hidden
T = 101770
NCORES = 8
TS = 12800      # per-core T shard (8*TS = 102400 >= T, zero padded)
SUP = 2048      # supertile columns (4 col-groups x 512)
NSUB = 512
KFE = 896       # 784 padded to 7*128
PB = NP * B     # 32
KA = FEAT + NP  # 68: att stream rows (feats part + rank-4 embeds/bias fold)
KG = GH + 1     # 33: gen stream rows (weights + bias)

ATT_FP8 = True
SC_W = 128.0    # host scale on att stream values
SC_F = 16.0     # device scale on feats in the att stationary
SC_OH = 16.0    # onehot value (matches SC_F so F rows align with A1 rows)
ABLATE = "full"  # "full" | "dma" | "compute"  (test-only knob)
STAGGERED = True   # staggered_reset on the repeats loop (timing-only knob)
RINGS = "2way"   # "2way" (att:sync, gen:scalar) | "3way" (+gpsimd)
GEN_MIXED = False  # gen matrix fp8 (x128) + bias row bf16; lgen stays bf16
OUT_BF16 = True    # write the output in bf16 (halves output DMA bytes)
GEN_COMB = True    # 4 col-groups folded into K: one K=128 G matmul + one
                   # K=4 bias matmul per (head, supertile); implies mixed
                   # dtypes (fp8 moving x bf16 stationary) for the G matmul

# ---------------------------------------------------------------------------
# Workaround: this container's walrus build rejects more than one sync-wait
# command per instruction, while Tile freely attaches several. Split the
# extra waits onto same-engine NoOps inserted just before the instruction
# (same semantics: the engine's sequencer blocks on each wait in order).
# ---------------------------------------------------------------------------
_orig_compile_bir_kernel = _bass_utils.compile_bir_kernel


def _split_multi_waits(bir):
    for fn in bir.get("functions", []):
        for bb in fn.get("blocks", []):
            out = []
            for ins in bb.get("instructions", []):
                si = ins.get("sync_info")
                waits = (si or {}).get("on_wait") or []
                if len(waits) > 1:
                    for k, w in enumerate(waits[:-1]):
                        out.append({
                            "debug": ins.get("debug", 0),
                            "engine": ins["engine"],
                            "ins": [],
                            "name": f"{ins['name']}-wsplit{k}",
                            "opcode": "NoOp",
                            "outs": [],
                            "sync_info": {"on_update": [], "on_wait": [w]},
                        })
                    si["on_wait"] = [waits[-1]]
                out.append(ins)
            bb["instructions"] = out
    return bir


def _patched_compile_bir_kernel(bir_json, tmpdir, neff_name="file.neff"):
    bir = _split_multi_waits(json.loads(bir_json))
    return _orig_compile_bir_kernel(json.dumps(bir).encode(), tmpdir,
                                    neff_name=neff_name)


def _install_patch():
    _bass_utils.compile_bir_kernel = _patched_compile_bir_kernel
    _bass2jax.compile_bir_kernel = _patched_compile_bir_kernel


_install_patch()


# ---------------------------------------------------------------------------
# Device program
# ---------------------------------------------------------------------------
def _build_bass(ts=TS, repeats=1, att_fp8=ATT_FP8):
    att_dt = F8 if att_fp8 else BF16
    nc = bass.Bass()

    att_in = nc.dram_tensor("att_in", [H, KA, ts], att_dt, kind="ExternalInput")
    if GEN_COMB:
        gen_in = nc.dram_tensor("gen_in", [H, 4 * GH, ts // 4], F8,
                                kind="ExternalInput")
        genb_in = nc.dram_tensor("genb_in", [H, 1, ts], BF16,
                                 kind="ExternalInput")
        i4sel = nc.dram_tensor("i4sel", [PB, 4 * 128], BF16,
                               kind="ExternalInput")
        sel4g = lg4scr = None
    elif GEN_MIXED:
        gen_in = nc.dram_tensor("gen_in", [H, GH, ts], F8, kind="ExternalInput")
        genb_in = nc.dram_tensor("genb_in", [H, 1, ts], BF16,
                                 kind="ExternalInput")
        i4sel = sel4g = lg4scr = None
    else:
        gen_in = nc.dram_tensor("gen_in", [H, KG, ts], BF16,
                                kind="ExternalInput")
        genb_in = i4sel = sel4g = lg4scr = None
    xt = nc.dram_tensor("xt", [KFE, B], BF16, kind="ExternalInput")
    fe1t = nc.dram_tensor("fe1t", [KFE, 128], BF16, kind="ExternalInput")
    fb1 = nc.dram_tensor("fb1", [128, 1], F32, kind="ExternalInput")
    fw2t = nc.dram_tensor("fw2t", [128, FEAT], BF16, kind="ExternalInput")
    fb2 = nc.dram_tensor("fb2", [FEAT, 1], F32, kind="ExternalInput")
    gwt = nc.dram_tensor("gwt", [FEAT + 1, H], BF16, kind="ExternalInput")
    emb = nc.dram_tensor("emb", [EMB, PB], BF16, kind="ExternalInput")
    sel4 = nc.dram_tensor("sel4", [B, PB], BF16, kind="ExternalInput")
    g1in = nc.dram_tensor("g1in", [HIN + 1, H * GH], BF16, kind="ExternalInput")
    oh16 = nc.dram_tensor("oh16", [NP, PB], att_dt, kind="ExternalInput")
    out = nc.dram_tensor("out", [PB, ts], BF16 if OUT_BF16 else F32,
                         kind="ExternalOutput")

    n_sup = ts // SUP  # full supertiles; plus one 512-wide tail
    assert ts == n_sup * SUP + NSUB

    with tile.TileContext(nc) as tc:
        with (
            tc.tile_pool(name="const", bufs=1) as cp,
            tc.tile_pool(name="stream", bufs=3) as sp,
            tc.tile_pool(name="psum", bufs=2, space="PSUM") as pp,
            tc.tile_pool(name="prepsum", bufs=1, space="PSUM") as prep,
            tc.tile_pool(name="ev", bufs=3) as ev,
            tc.tile_pool(name="accp", bufs=2) as accp,
        ):
            def body():
                _emit_iter(nc, tc, cp, sp, pp, prep, ev, accp,
                           att_in, gen_in, genb_in, i4sel, sel4g, lg4scr,
                           xt, fe1t, fb1, fw2t, fb2, gwt, emb, sel4, g1in,
                           oh16, out, n_sup)

            if repeats > 1:
                with tc.For_i(0, repeats,
                              staggered_reset=STAGGERED,
                              hint_engines=(mybir.EngineType.PE,
                                            mybir.EngineType.SP,
                                            mybir.EngineType.DVE,
                                            mybir.EngineType.Activation)):
                    body()
            else:
                body()
    return nc


def _emit_iter(nc, tc, cp, sp, pp, prep, ev, accp,
               att_in, gen_in, genb_in, i4sel, sel4g, lg4scr, xt, fe1t, fb1,
               fw2t, fb2, gwt, emb, sel4, g1in, oh16, out, n_sup):
    att_dt = att_in.dtype
    fp8 = att_dt == F8
    sc_f = SC_F if fp8 else 1.0
    inv_scale = 1.0 / (SC_W * sc_f) if fp8 else 1.0

    # ---- constant loads ---------------------------------------------------
    fe1_t = cp.tile([128, 7, 128], BF16)
    nc.sync.dma_start(fe1_t[:], fe1t.rearrange("(o p) m -> p o m", p=128))
    xt_t = cp.tile([128, 7, B], BF16)
    nc.sync.dma_start(xt_t[:], xt.rearrange("(o p) m -> p o m", p=128))
    fb1_t = cp.tile([128, 1], F32)
    nc.sync.dma_start(fb1_t[:], fb1[:])
    fw2_t = cp.tile([128, FEAT], BF16)
    nc.sync.dma_start(fw2_t[:], fw2t[:])
    fb2_t = cp.tile([FEAT, 1], F32)
    nc.sync.dma_start(fb2_t[:], fb2[:])
    gwt_t = cp.tile([FEAT + 1, H], BF16)
    nc.sync.dma_start(gwt_t[:], gwt[:])
    sel4_t = cp.tile([B, PB], BF16)
    nc.sync.dma_start(sel4_t[:], sel4[:])
    g1_t = cp.tile([HIN + 1, H * GH], BF16)
    nc.sync.dma_start(g1_t[:], g1in[:])

    hinT = cp.tile([HIN + 1, PB], BF16)     # [97, 32] stationary (gen_W1)
    hinF = cp.tile([KA, PB], att_dt)        # [68, 32] stationary (att)
    if GEN_COMB:
        # block-diagonal gen stationary [128, 128] per head, plus its K=4
        # gate/bias companion [4, 128] per head (built at partitions
        # {0,32,64,96} and moved to partitions 0..3 via a DRAM round-trip,
        # since engine APs must start at a 32-multiple partition)
        lgen2 = cp.tile([128, H * 128], BF16, name="lgen2", tag="lgen2")
        i4_t = cp.tile([PB, 4 * 128], BF16, name="i4_t", tag="i4_t")
        nc.sync.dma_start(i4_t[:], i4sel[:])
        lgen = None
    else:
        lgen = cp.tile([KG, H * PB], BF16)  # [33, 8*32] stationary (gen)
        lgen2 = None
    # partition-0 gate row (the K=1 bias matmuls need their stationary to
    # start at the same partition as the moving operand)
    lgenb = (cp.tile([1, H * PB], BF16, name="lgenb", tag="lgenb")
             if (GEN_MIXED or GEN_COMB) else None)

    # ---- feature extractor ------------------------------------------------
    psf = prep.tile([128, 32], F32, tag="pre1")
    for o in range(7):
        nc.tensor.matmul(psf[:, :B], fe1_t[:, o, :], xt_t[:, o, :],
                         start=(o == 0), stop=(o == 6))
    relu1 = cp.tile([128, B], BF16)
    nc.scalar.activation(relu1[:], psf[:, :B], AF.Relu, bias=fb1_t[:])

    psf2 = prep.tile([128, 32], F32, tag="pre1")
    nc.tensor.matmul(psf2[:FEAT, :B], fw2_t[:], relu1[:],
                     start=True, stop=True)
    featsT = cp.tile([FEAT + 1, B], BF16)   # [65, 8], row 64 = ones
    nc.scalar.activation(featsT[:FEAT, :], psf2[:FEAT, :B], AF.Identity,
                         bias=fb2_t[:])
    nc.vector.memset(featsT[FEAT:FEAT + 1, :], 1.0)

    # ---- head gate (softmax over heads, normalization folded) -------------
    psgl = prep.tile([128, 32], F32, tag="pre1")
    nc.tensor.matmul(psgl[:B, :B], featsT[:], gwt_t[:],
                     start=True, stop=True)
    gateb = cp.tile([32, 32], F32)          # gate[b, h] in [0:8, 0:8]
    nc.vector.memset(gateb[:], 0.0)
    nc.scalar.activation(gateb[:B, :B], psgl[:B, :B], AF.Exp)
    sums = cp.tile([B, 1], F32)
    nc.vector.tensor_reduce(sums[:], gateb[:B, :B], AX.X, ALU.add)
    recip = cp.tile([B, 1], F32)
    nc.vector.reciprocal(recip[:], sums[:])
    nc.vector.tensor_scalar_mul(gateb[:B, :B], gateb[:B, :B], recip[:])
    gatebT = cp.tile([32, 32], F32)         # gate[h, b] in [0:8, 0:8]
    nc.vector.transpose(gatebT[:], gateb[:])
    gatebT_bf = cp.tile([32, 32], BF16)
    nc.vector.tensor_copy(gatebT_bf[:], gatebT[:])
    # gate column per (pb, h): gcols[pb, h] = gate[h, pb % 8]
    psgc = prep.tile([128, 32], F32, tag="pre1")
    nc.tensor.matmul(psgc[:PB, :B], sel4_t[:], gatebT_bf[:B, :B],
                     start=True, stop=True)
    gcols = cp.tile([PB, B], F32)
    nc.vector.tensor_copy(gcols[:], psgc[:PB, :B])

    # ---- hinT (stationary for the gen_W1 matmuls) -------------------------
    for p in range(NP):
        nc.vector.tensor_copy(hinT[:FEAT, p * B:(p + 1) * B],
                              featsT[:FEAT, :])
    nc.sync.dma_start(hinT[FEAT:HIN, :], emb[:])
    nc.vector.memset(hinT[HIN:HIN + 1, :], 1.0)

    # ---- hinF (stationary of the att matmuls): feats x SC_F, onehot -------
    for p in range(NP):
        nc.scalar.mul(hinF[:FEAT, p * B:(p + 1) * B], featsT[:FEAT, :],
                      float(sc_f))
    nc.sync.dma_start(hinF[FEAT:KA, :], oh16[:])

    # ---- per-head gen stationary operand ----------------------------------
    if not GEN_COMB:
        tmp32 = cp.tile([PB, GH], F32, tag="tmpT")
    for h in range(H):
        psh = prep.tile([128, 32], F32, tag="preh")
        nc.tensor.matmul(psh[:PB, :GH], hinT[:], g1_t[:, h * GH:(h + 1) * GH],
                         start=True, stop=True)
        hmid = cp.tile([PB, GH], F32, tag="hmid")
        nc.scalar.activation(hmid[:], psh[:PB, :GH], AF.Relu)
        nc.vector.tensor_scalar_mul(hmid[:], hmid[:], gcols[:, h:h + 1])
        if GEN_COMB:
            hmid_bf = cp.tile([PB, GH], BF16, name="hmid_bf", tag="hmid_bf")
            nc.scalar.copy(hmid_bf[:], hmid[:])
            # lgen2 block-diag: out[32g+j, 32g'+pb] = d(g,g')*hmid[pb, j]
            psLG = prep.tile([128, 128], F32, tag="psLG")
            for g in range(4):
                nc.tensor.matmul(psLG[32 * g:32 * (g + 1), :], hmid_bf[:],
                                 i4_t[:, g * 128:(g + 1) * 128],
                                 start=True, stop=True,
                                 tile_position=(0, 32 * g))
            nc.vector.tensor_copy(lgen2[:, h * 128:(h + 1) * 128], psLG[:])
            nc.tensor.matmul(psh[GH:GH + 1, :PB], gatebT_bf[:B, h:h + 1],
                             sel4_t[:], start=True, stop=True,
                             tile_position=(0, 32))
            nc.scalar.mul(lgenb[:, h * PB:(h + 1) * PB],
                          psh[GH:GH + 1, :PB], float(SC_W))
            continue
        nc.vector.transpose(tmp32[:], hmid[:])
        nc.vector.tensor_copy(lgen[:GH, h * PB:(h + 1) * PB], tmp32[:])
        nc.tensor.matmul(psh[GH:GH + 1, :PB], gatebT_bf[:B, h:h + 1],
                         sel4_t[:], start=True, stop=True,
                         tile_position=(0, 32))
        if GEN_MIXED:
            nc.scalar.mul(lgenb[:, h * PB:(h + 1) * PB],
                          psh[GH:GH + 1, :PB], float(SC_W))
        else:
            nc.vector.tensor_copy(lgen[GH:GH + 1, h * PB:(h + 1) * PB],
                                  psh[GH:GH + 1, :PB])

    # ---- main streamed loop: whole-head DMAs, supertile compute -----------
    ts = out.shape[1]
    acc = accp.tile([128, ts // 4], F32, tag="acc")
    if OUT_BF16:
        accout = accp.tile([128, ts // 4], BF16, name="accout", tag="accb")
    else:
        accout = acc
    if ABLATE == "dma":
        nc.vector.memset(accout[:], 0.0)
    if RINGS == "3way":
        att_eng = [nc.sync] * 6 + [nc.scalar] * 2
        gen_eng = [nc.scalar] * 4 + [nc.gpsimd] * 4
        out_eng = nc.sync
    elif RINGS == "1way":
        att_eng = [nc.sync] * H
        gen_eng = [nc.sync] * H
        out_eng = nc.sync
    elif RINGS == "2bal":
        att_eng = [nc.sync] * H
        gen_eng = [nc.scalar] * H
        out_eng = nc.scalar
    elif RINGS == "2alt":
        att_eng = [nc.sync if h % 2 == 0 else nc.scalar for h in range(H)]
        gen_eng = [nc.scalar if h % 2 == 0 else nc.sync for h in range(H)]
        out_eng = nc.scalar
    else:
        att_eng = [nc.sync] * H
        gen_eng = [nc.scalar] * H
        out_eng = nc.sync
    first = {}
    for h in range(H):
        if ABLATE != "compute" or h == 0:
            att_t = sp.tile([KA, ts], att_dt, tag="att")
            att_eng[h].dma_start(att_t[:], att_in[h])
            if GEN_COMB:
                gen_t = sp.tile([4 * GH, ts // 4], F8, tag="gen")
                gen_eng[h].dma_start(gen_t[:], gen_in[h])
                genb_t = sp.tile([1, ts], BF16, tag="genb")
                gen_eng[h].dma_start(genb_t[:], genb_in[h])
            elif GEN_MIXED:
                gen_t = sp.tile([GH, ts], F8, tag="gen")
                gen_eng[h].dma_start(gen_t[:], gen_in[h])
                genb_t = sp.tile([1, ts], BF16, tag="genb")
                gen_eng[h].dma_start(genb_t[:], genb_in[h])
            else:
                gen_t = sp.tile([KG, ts], BF16, tag="gen")
                gen_eng[h].dma_start(gen_t[:], gen_in[h])
                genb_t = None
            first.setdefault("att", att_t)
            first.setdefault("gen", gen_t)
            first.setdefault("genb", genb_t)
        else:
            att_t, gen_t, genb_t = first["att"], first["gen"], first["genb"]
        if ABLATE == "dma":
            continue
        for s in range(n_sup + 1):
            ncols = SUP if s < n_sup else NSUB
            ns = ncols // 4
            c0 = s * SUP
            psA = pp.tile([128, NSUB], F32, tag="psA")
            psG = pp.tile([128, NSUB], F32, tag="psG")
            for g in range(4):
                nc.tensor.matmul(psA[32 * g:32 * (g + 1), :ns], hinF[:],
                                 att_t[:, c0 + g * ns:c0 + (g + 1) * ns],
                                 start=True, stop=True,
                                 tile_position=(0, 32 * g))
            if GEN_COMB:
                q0 = s * NSUB
                nc.tensor.matmul(psG[:, :ns],
                                 lgen2[:, h * 128:(h + 1) * 128],
                                 gen_t[:, q0:q0 + ns],
                                 start=True, stop=False)
                for g in range(4):
                    nc.tensor.matmul(psG[32 * g:32 * (g + 1), :ns],
                                     lgenb[:, h * PB:(h + 1) * PB],
                                     genb_t[:, c0 + g * ns:c0 + (g + 1) * ns],
                                     start=False, stop=True,
                                     tile_position=(0, 32 * g))
            elif GEN_MIXED:
                for g in range(4):
                    nc.tensor.matmul(psG[32 * g:32 * (g + 1), :ns],
                                     lgen[:GH, h * PB:(h + 1) * PB],
                                     gen_t[:, c0 + g * ns:c0 + (g + 1) * ns],
                                     start=True, stop=False,
                                     tile_position=(0, 32 * g))
                for g in range(4):
                    nc.tensor.matmul(psG[32 * g:32 * (g + 1), :ns],
                                     lgenb[:, h * PB:(h + 1) * PB],
                                     genb_t[:, c0 + g * ns:c0 + (g + 1) * ns],
                                     start=False, stop=True,
                                     tile_position=(0, 32 * g))
            else:
                for g in range(4):
                    nc.tensor.matmul(psG[32 * g:32 * (g + 1), :ns],
                                     lgen[:, h * PB:(h + 1) * PB],
                                     gen_t[:, c0 + g * ns:c0 + (g + 1) * ns],
                                     start=True, stop=True,
                                     tile_position=(0, 32 * g))
            imp = ev.tile([128, NSUB], F32, tag="imp")
            nc.scalar.activation(imp[:, :ns], psA[:, :ns], AF.Sigmoid,
                                 scale=float(inv_scale))
            a0 = s * NSUB
            gsc = 1.0 / SC_W if (GEN_MIXED or GEN_COMB) else 1.0
            if h == 0:
                nc.vector.scalar_tensor_tensor(acc[:, a0:a0 + ns],
                                               psG[:, :ns], float(gsc),
                                               imp[:, :ns],
                                               ALU.mult, ALU.mult)
            else:
                tmp = ev.tile([128, NSUB], F32, tag="tmp")
                nc.vector.scalar_tensor_tensor(tmp[:, :ns],
                                               psG[:, :ns], float(gsc),
                                               imp[:, :ns],
                                               ALU.mult, ALU.mult)
                dst = accout if (OUT_BF16 and h == H - 1) else acc
                nc.vector.tensor_add(dst[:, a0:a0 + ns], acc[:, a0:a0 + ns],
                                     tmp[:, :ns])
    for s in range(n_sup + 1):
        ncols = SUP if s < n_sup else NSUB
        ns = ncols // 4
        out_eng.dma_start(
            out[:, s * SUP:s * SUP + ncols].rearrange("p (g c) -> g p c", g=4),
            accout[:, s * NSUB:s * NSUB + ns])
    return nc


_NC_CACHE = None


def _get_nc():
    global _NC_CACHE
    if _NC_CACHE is None:
        _NC_CACHE = _build_bass()
    return _NC_CACHE


# ---------------------------------------------------------------------------
# Host wrapper
# ---------------------------------------------------------------------------
LAST_RESULTS = None  # BassKernelResults of the last run (for profiling)
LAST_IN_MAPS = None  # per-core input maps of the last run (for benchmarking)


def kernel(x, fe_W1, fe_b1, fe_W2, fe_b2, embeds,
           gen_W1, gen_b1, gen_W2, gen_b2, att_W, att_b,
           gate_W, gate_b):
    f32 = np.float32
    np_att = mybir.dt.np(F8 if ATT_FP8 else BF16)
    np_bf = mybir.dt.np(BF16)
    x = np.asarray(x, f32)
    fe_W1 = np.asarray(fe_W1, f32)
    fe_b1 = np.asarray(fe_b1, f32)
    fe_W2 = np.asarray(fe_W2, f32)
    fe_b2 = np.asarray(fe_b2, f32)
    embeds = np.asarray(embeds, f32)
    gen_W1 = np.asarray(gen_W1, f32)
    gen_b1 = np.asarray(gen_b1, f32)
    gen_W2 = np.asarray(gen_W2, f32)
    gen_b2 = np.asarray(gen_b2, f32)
    att_W = np.asarray(att_W, f32)
    att_b = np.asarray(att_b, f32)
    gate_W = np.asarray(gate_W, f32)
    gate_b = np.asarray(gate_b, f32)

    sc_w = SC_W if ATT_FP8 else 1.0
    # --- att stream: [H, 68, T_pad] (feats rows + rank-4 embeds/bias fold) --
    tpad = NCORES * TS
    att_all = np.zeros((H, KA, tpad), np_att)
    att_all[:, :FEAT, :T] = (att_W[:, :, :FEAT].transpose(0, 2, 1)
                             * sc_w).astype(np_att)
    F = np.einsum('pj,htj->hpt', embeds, att_W[:, :, FEAT:]) \
        + att_b[:, None, :]
    att_all[:, FEAT:, :T] = (F * sc_w).astype(np_att)
    # --- gen stream ---------------------------------------------------------
    if GEN_COMB:
        G8 = np.zeros((H, GH, tpad), np.float32)
        G8[:, :, :T] = gen_W2.transpose(0, 2, 1) * SC_W
        gen_all = G8  # regrouped per-core below
        genb_all = np.zeros((H, 1, tpad), np_bf)
        genb_all[:, 0, :T] = gen_b2.astype(np_bf)
    elif GEN_MIXED:
        np_f8 = mybir.dt.np(F8)
        gen_all = np.zeros((H, GH, tpad), np_f8)
        gen_all[:, :, :T] = (gen_W2.transpose(0, 2, 1) * SC_W).astype(np_f8)
        genb_all = np.zeros((H, 1, tpad), np_bf)
        genb_all[:, 0, :T] = gen_b2.astype(np_bf)
    else:
        gen_all = np.zeros((H, KG, tpad), np_bf)
        gen_all[:, :GH, :T] = gen_W2.transpose(0, 2, 1).astype(np_bf)
        gen_all[:, GH, :T] = gen_b2.astype(np_bf)
        genb_all = None

    # --- small shared operands ---
    xt = np.zeros((KFE, B), np_bf)
    xt[:784] = x.T.astype(np_bf)
    fe1t = np.zeros((KFE, 128), np_bf)
    fe1t[:784] = fe_W1.T.astype(np_bf)
    fb1 = np.ascontiguousarray(fe_b1[:, None])
    fw2t = np.ascontiguousarray(fe_W2.T.astype(np_bf))
    fb2 = np.ascontiguousarray(fe_b2[:, None])
    gwt = np.concatenate([gate_W.T, gate_b[None, :]], axis=0).astype(np_bf)
    emb = np.repeat(embeds.T[:, :, None], B, axis=2).reshape(EMB, PB)
    sel4 = np.tile(np.eye(B, dtype=f32), NP)
    g1in = np.concatenate([gen_W1.transpose(0, 2, 1), gen_b1[:, None, :]],
                          axis=1)                      # [H, 97, 32]
    g1in = g1in.transpose(1, 0, 2).reshape(HIN + 1, H * GH)
    oh = np.repeat(np.eye(NP, dtype=f32), B, axis=1) * (SC_OH if ATT_FP8
                                                        else 1.0)
    # selector constant for the combined-gen stationary build
    i4sel_a = np.zeros((PB, 4 * 128), f32)
    for g in range(4):
        for pb in range(PB):
            i4sel_a[pb, g * 128 + 32 * g + pb] = 1.0

    shared = {
        "xt": xt, "fe1t": fe1t, "fb1": fb1, "fw2t": fw2t, "fb2": fb2,
        "gwt": np.ascontiguousarray(gwt),
        "emb": np.ascontiguousarray(emb.astype(np_bf)),
        "sel4": np.ascontiguousarray(sel4.astype(np_bf)),
        "g1in": np.ascontiguousarray(g1in.astype(np_bf)),
        "oh16": np.ascontiguousarray(oh.astype(np_att)),
    }
    if GEN_COMB:
        shared["i4sel"] = np.ascontiguousarray(i4sel_a.astype(np_bf))

    def _regroup(a):
        # [H, k, TS] -> [H, 4k, TS//4]: column group g of each supertile
        # becomes contraction rows [g*k, (g+1)*k)
        k = a.shape[1]
        o = np.zeros((H, 4 * k, TS // 4), a.dtype)
        for s in range(7):
            c0 = s * SUP
            ns = (SUP if s < 6 else NSUB) // 4
            for g in range(4):
                o[:, g * k:(g + 1) * k, s * NSUB:s * NSUB + ns] = \
                    a[:, :, c0 + g * ns:c0 + (g + 1) * ns]
        return o

    in_maps = []
    for c in range(NCORES):
        sl = slice(c * TS, (c + 1) * TS)
        m = dict(shared)
        m["att_in"] = np.ascontiguousarray(att_all[:, :, sl])
        if GEN_COMB:
            np_f8 = mybir.dt.np(F8)
            m["gen_in"] = np.ascontiguousarray(
                _regroup(gen_all[:, :, sl]).astype(np_f8))
            m["genb_in"] = np.ascontiguousarray(genb_all[:, :, sl])
        else:
            m["gen_in"] = np.ascontiguousarray(gen_all[:, :, sl])
            if genb_all is not None:
                m["genb_in"] = np.ascontiguousarray(genb_all[:, :, sl])
        in_maps.append(m)

    nc = _get_nc()
    res = run_bass_kernel_spmd(nc, in_maps, core_ids=list(range(NCORES)))
    global LAST_RESULTS, LAST_IN_MAPS
    LAST_RESULTS = res
    LAST_IN_MAPS = in_maps

    full = np.concatenate(
        [np.asarray(res.results[c]["out"], np.float32) for c in range(NCORES)],
        axis=1)[:, :T]                                # [32, T], row = p*8+b
    return np.ascontiguousarray(
        full.reshape(NP, B, T).transpose(1, 0, 2).reshape(B, NP * T))


# ---------------------------------------------------------------------------
# Timing harness (test-only): device-resident inputs, repeated execution.
# ---------------------------------------------------------------------------
def benchmark_last(in_maps, iters=8, nc=None):
    import time

    import jax
    from concourse import bass2jax as b2j
    from concourse import mybir as _mybir

    if nc is None:
        nc = _get_nc()
    b2j.install_neuronx_cc_hook()

    partition_name = (nc.partition_id_tensor.name
                      if nc.partition_id_tensor else None)
    in_names, out_names, out_avals, zero_outs = [], [], [], []
    for alloc in nc.m.functions[0].allocations:
        if not isinstance(alloc, _mybir.MemoryLocationSet):
            continue
        name = alloc.memorylocations[0].name
        if alloc.kind == "ExternalInput":
            if name != partition_name:
                in_names.append(name)
        elif alloc.kind == "ExternalOutput":
            shape = tuple(alloc.tensor_shape)
            dtype = _mybir.dt.np(alloc.dtype)
            out_names.append(name)
            out_avals.append(jax.core.ShapedArray(shape, dtype))
            zero_outs.append(np.zeros(shape, dtype))
    n_params = len(in_names)
    n_outs = len(out_avals)
    in_names_all = in_names + out_names
    if partition_name is not None:
        in_names_all.append(partition_name)

    def _body(*args):
        operands = list(args)
        if partition_name is not None:
            operands.append(b2j.partition_id_tensor())
        return tuple(b2j._bass_exec_p.bind(
            *operands,
            out_avals=tuple(out_avals),
            in_names=tuple(in_names_all),
            out_names=tuple(out_names),
            lowering_input_output_aliases=(),
            sim_require_finite=True,
            sim_require_nnan=True,
            nc=nc,
        ))

    donate = tuple(range(n_params, n_params + n_outs))
    devices = jax.devices()[:NCORES]
    mesh = b2j.Mesh(np.asarray(devices), ("core",))
    sharded = jax.jit(
        b2j.shard_map(_body, mesh=mesh,
                      in_specs=(b2j.PartitionSpec("core"),) * (n_params + n_outs),
                      out_specs=(b2j.PartitionSpec("core"),) * n_outs,
                      check_rep=False),
        donate_argnums=donate, keep_unused=True)

    concat_in = [
        np.concatenate([np.asarray(in_maps[c][nm]) for c in range(NCORES)],
                       axis=0)
        for nm in in_names
    ]
    sharding = jax.sharding.NamedSharding(mesh, b2j.PartitionSpec("core"))
    dev_in = [jax.device_put(a, sharding) for a in concat_in]

    def _zeros():
        return [jax.device_put(
            np.zeros((NCORES * z.shape[0], *z.shape[1:]), z.dtype), sharding)
            for z in zero_outs]

    # warmup (compile + load)
    outs = sharded(*dev_in, *_zeros())
    jax.block_until_ready(outs)
    times = []
    for _ in range(iters):
        zs = _zeros()
        jax.block_until_ready(zs)
        t0 = time.perf_counter()
        outs = sharded(*dev_in, *zs)
        jax.block_until_ready(outs)
        times.append(time.perf_counter() - t0)
    return min(times), times



# revision 1
# speedup vs baseline: 1.6962x; 1.6962x over previous
"""Trainium2 Bass kernel for nn_DynamicSelectiveHyperNet.

Strategy
--------
Shard the target-parameter axis T across the 8 NeuronCores (no collectives;
the gated head-sum is computed locally per T-slice). Each core runs all 8
heads for its slice. The kernel is HBM-byte-bound, so the streamed weights
are compressed:

  att stream  [H, 68, ts]   fp8-e4m3, values x128:
      rows 0..63  = att_W[h, t, 0:64].T          (feats part)
      rows 64..67 = F[h, p, t] = embeds[p] . att_W[h, t, 64:96] + att_b[h, t]
      (host-precomputed, weights-only: the rank-4 embeds block + bias fold
       96+1 contraction rows down to 4)
  gen stream  [H, 128, ts/4] fp8-e4m3 x128: gen_W2 with the 4 column groups
      of each 2048-wide supertile folded into the contraction dim, so one
      K=128 matmul (vs 4) covers a supertile; bf16 block-diagonal stationary.
  gen bias    [H, 1, ts]    bf16 (fp8 bias fails the 2e-2 gate)

Per (head, supertile): 4x K=68 fp8 att matmuls (logits x2048, un-scaled by
the sigmoid's ACT scale), 1x K=128 mixed fp8/bf16 gen matmul, 4x K=1 bias
matmuls accumulating into the same PSUM; sigmoid on ACT; fused
scale-multiply + add on DVE. Output is written in bf16 (harness-side cast
back to fp32). Measured rel err (absmax-normalized): 1.33e-2 vs the 2e-2
gate; all-fp8 gen (2.1e-2+) and fp8 bias (2.7e-2) fail, bf16-gen (4.6e-3)
is the fallback via GEN_COMB=GEN_MIXED=False.

The preamble (feature extractor, gate softmax, per-head hmid, stationaries)
runs inside the repeat loop used for timing, so amortized per-iteration
numbers include it. DMA rings: att+consts+out on SP, gen on Activation
(gpsimd SWDGE DMAs crash this walrus build; bandwidth is shared anyway).
"""

import sys

sys.path.insert(0, "/opt/trn_rl_repo")

import json

import numpy as np

import concourse.bass as bass
import concourse.bass2jax as _bass2jax
import concourse.bass_utils as _bass_utils
import concourse.tile as tile
from concourse import mybir
from concourse.bass_utils import run_bass_kernel_spmd

AF = mybir.ActivationFunctionType
ALU = mybir.AluOpType
F32 = mybir.dt.float32
BF16 = mybir.dt.bfloat16
F8 = mybir.dt.float8e4
AX = mybir.AxisListType

B = 8
H = 8
NP = 4          # target param groups
FEAT = 64
EMB = 32
HIN = 96        # FEAT + EMB
GH = 32         # generator hidden
T = 101770
NCORES = 8
TS = 12800      # per-core T shard (8*TS = 102400 >= T, zero padded)
SUP = 2048      # supertile columns (4 col-groups x 512)
NSUB = 512
KFE = 896       # 784 padded to 7*128
PB = NP * B     # 32
KA = FEAT + NP  # 68: att stream rows (feats part + rank-4 embeds/bias fold)
KG = GH + 1     # 33: gen stream rows (weights + bias)

ATT_FP8 = True
SC_W = 128.0    # host scale on att stream values
SC_F = 16.0     # device scale on feats in the att stationary
SC_OH = 16.0    # onehot value (matches SC_F so F rows align with A1 rows)
ABLATE = "full"  # "full" | "dma" | "compute"  (test-only knob)
STAGGERED = True   # staggered_reset on the repeats loop (timing-only knob)
RINGS = "2way"   # "2way" (att:sync, gen:scalar) | "3way" (+gpsimd)
GEN_MIXED = False  # gen matrix fp8 (x128) + bias row bf16; lgen stays bf16
OUT_BF16 = True    # write the output in bf16 (halves output DMA bytes)
GEN_COMB = True    # 4 col-groups folded into K: one K=128 G matmul + one
                   # K=4 bias matmul per (head, supertile); implies mixed
                   # dtypes (fp8 moving x bf16 stationary) for the G matmul

# ---------------------------------------------------------------------------
# Workaround: this container's walrus build rejects more than one sync-wait
# command per instruction, while Tile freely attaches several. Split the
# extra waits onto same-engine NoOps inserted just before the instruction
# (same semantics: the engine's sequencer blocks on each wait in order).
# ---------------------------------------------------------------------------
_orig_compile_bir_kernel = _bass_utils.compile_bir_kernel


def _split_multi_waits(bir):
    for fn in bir.get("functions", []):
        for bb in fn.get("blocks", []):
            out = []
            for ins in bb.get("instructions", []):
                si = ins.get("sync_info")
                waits = (si or {}).get("on_wait") or []
                if len(waits) > 1:
                    for k, w in enumerate(waits[:-1]):
                        out.append({
                            "debug": ins.get("debug", 0),
                            "engine": ins["engine"],
                            "ins": [],
                            "name": f"{ins['name']}-wsplit{k}",
                            "opcode": "NoOp",
                            "outs": [],
                            "sync_info": {"on_update": [], "on_wait": [w]},
                        })
                    si["on_wait"] = [waits[-1]]
                out.append(ins)
            bb["instructions"] = out
    return bir


def _patched_compile_bir_kernel(bir_json, tmpdir, neff_name="file.neff"):
    bir = _split_multi_waits(json.loads(bir_json))
    return _orig_compile_bir_kernel(json.dumps(bir).encode(), tmpdir,
                                    neff_name=neff_name)


def _install_patch():
    _bass_utils.compile_bir_kernel = _patched_compile_bir_kernel
    _bass2jax.compile_bir_kernel = _patched_compile_bir_kernel


_install_patch()


# ---------------------------------------------------------------------------
# Device program
# ---------------------------------------------------------------------------
def _build_bass(ts=TS, repeats=1, att_fp8=ATT_FP8):
    att_dt = F8 if att_fp8 else BF16
    nc = bass.Bass()

    att_in = nc.dram_tensor("att_in", [H, KA, ts], att_dt, kind="ExternalInput")
    if GEN_COMB:
        gen_in = nc.dram_tensor("gen_in", [H, 4 * GH, ts // 4], F8,
                                kind="ExternalInput")
        genb_in = nc.dram_tensor("genb_in", [H, 1, ts], BF16,
                                 kind="ExternalInput")
        i4sel = nc.dram_tensor("i4sel", [PB, 4 * 128], BF16,
                               kind="ExternalInput")
        sel4g = lg4scr = None
    elif GEN_MIXED:
        gen_in = nc.dram_tensor("gen_in", [H, GH, ts], F8, kind="ExternalInput")
        genb_in = nc.dram_tensor("genb_in", [H, 1, ts], BF16,
                                 kind="ExternalInput")
        i4sel = sel4g = lg4scr = None
    else:
        gen_in = nc.dram_tensor("gen_in", [H, KG, ts], BF16,
                                kind="ExternalInput")
        genb_in = i4sel = sel4g = lg4scr = None
    xt = nc.dram_tensor("xt", [KFE, B], BF16, kind="ExternalInput")
    fe1t = nc.dram_tensor("fe1t", [KFE, 128], BF16, kind="ExternalInput")
    fb1 = nc.dram_tensor("fb1", [128, 1], F32, kind="ExternalInput")
    fw2t = nc.dram_tensor("fw2t", [128, FEAT], BF16, kind="ExternalInput")
    fb2 = nc.dram_tensor("fb2", [FEAT, 1], F32, kind="ExternalInput")
    gwt = nc.dram_tensor("gwt", [FEAT + 1, H], BF16, kind="ExternalInput")
    emb = nc.dram_tensor("emb", [EMB, PB], BF16, kind="ExternalInput")
    sel4 = nc.dram_tensor("sel4", [B, PB], BF16, kind="ExternalInput")
    g1in = nc.dram_tensor("g1in", [HIN + 1, H * GH], BF16, kind="ExternalInput")
    oh16 = nc.dram_tensor("oh16", [NP, PB], att_dt, kind="ExternalInput")
    out = nc.dram_tensor("out", [PB, ts], BF16 if OUT_BF16 else F32,
                         kind="ExternalOutput")

    n_sup = ts // SUP  # full supertiles; plus one 512-wide tail
    assert ts == n_sup * SUP + NSUB

    with tile.TileContext(nc) as tc:
        with (
            tc.tile_pool(name="const", bufs=1) as cp,
            tc.tile_pool(name="stream", bufs=3) as sp,
            tc.tile_pool(name="psum", bufs=2, space="PSUM") as pp,
            tc.tile_pool(name="prepsum", bufs=1, space="PSUM") as prep,
            tc.tile_pool(name="ev", bufs=3) as ev,
            tc.tile_pool(name="accp", bufs=2) as accp,
        ):
            def body():
                _emit_iter(nc, tc, cp, sp, pp, prep, ev, accp,
                           att_in, gen_in, genb_in, i4sel, sel4g, lg4scr,
                           xt, fe1t, fb1, fw2t, fb2, gwt, emb, sel4, g1in,
                           oh16, out, n_sup)

            if repeats > 1:
                with tc.For_i(0, repeats,
                              staggered_reset=STAGGERED,
                              hint_engines=(mybir.EngineType.PE,
                                            mybir.EngineType.SP,
                                            mybir.EngineType.DVE,
                                            mybir.EngineType.Activation)):
                    body()
            else:
                body()
    return nc


def _emit_iter(nc, tc, cp, sp, pp, prep, ev, accp,
               att_in, gen_in, genb_in, i4sel, sel4g, lg4scr, xt, fe1t, fb1,
               fw2t, fb2, gwt, emb, sel4, g1in, oh16, out, n_sup):
    att_dt = att_in.dtype
    fp8 = att_dt == F8
    sc_f = SC_F if fp8 else 1.0
    inv_scale = 1.0 / (SC_W * sc_f) if fp8 else 1.0

    # ---- constant loads ---------------------------------------------------
    fe1_t = cp.tile([128, 7, 128], BF16)
    nc.sync.dma_start(fe1_t[:], fe1t.rearrange("(o p) m -> p o m", p=128))
    xt_t = cp.tile([128, 7, B], BF16)
    nc.sync.dma_start(xt_t[:], xt.rearrange("(o p) m -> p o m", p=128))
    fb1_t = cp.tile([128, 1], F32)
    nc.sync.dma_start(fb1_t[:], fb1[:])
    fw2_t = cp.tile([128, FEAT], BF16)
    nc.sync.dma_start(fw2_t[:], fw2t[:])
    fb2_t = cp.tile([FEAT, 1], F32)
    nc.sync.dma_start(fb2_t[:], fb2[:])
    gwt_t = cp.tile([FEAT + 1, H], BF16)
    nc.sync.dma_start(gwt_t[:], gwt[:])
    sel4_t = cp.tile([B, PB], BF16)
    nc.sync.dma_start(sel4_t[:], sel4[:])
    g1_t = cp.tile([HIN + 1, H * GH], BF16)
    nc.sync.dma_start(g1_t[:], g1in[:])

    hinT = cp.tile([HIN + 1, PB], BF16)     # [97, 32] stationary (gen_W1)
    hinF = cp.tile([KA, PB], att_dt)        # [68, 32] stationary (att)
    if GEN_COMB:
        # block-diagonal gen stationary [128, 128] per head, plus its K=4
        # gate/bias companion [4, 128] per head (built at partitions
        # {0,32,64,96} and moved to partitions 0..3 via a DRAM round-trip,
        # since engine APs must start at a 32-multiple partition)
        lgen2 = cp.tile([128, H * 128], BF16, name="lgen2", tag="lgen2")
        i4_t = cp.tile([PB, 4 * 128], BF16, name="i4_t", tag="i4_t")
        nc.sync.dma_start(i4_t[:], i4sel[:])
        lgen = None
    else:
        lgen = cp.tile([KG, H * PB], BF16)  # [33, 8*32] stationary (gen)
        lgen2 = None
    # partition-0 gate row (the K=1 bias matmuls need their stationary to
    # start at the same partition as the moving operand)
    lgenb = (cp.tile([1, H * PB], BF16, name="lgenb", tag="lgenb")
             if (GEN_MIXED or GEN_COMB) else None)

    # ---- feature extractor ------------------------------------------------
    psf = prep.tile([128, 32], F32, tag="pre1")
    for o in range(7):
        nc.tensor.matmul(psf[:, :B], fe1_t[:, o, :], xt_t[:, o, :],
                         start=(o == 0), stop=(o == 6))
    relu1 = cp.tile([128, B], BF16)
    nc.scalar.activation(relu1[:], psf[:, :B], AF.Relu, bias=fb1_t[:])

    psf2 = prep.tile([128, 32], F32, tag="pre1")
    nc.tensor.matmul(psf2[:FEAT, :B], fw2_t[:], relu1[:],
                     start=True, stop=True)
    featsT = cp.tile([FEAT + 1, B], BF16)   # [65, 8], row 64 = ones
    nc.scalar.activation(featsT[:FEAT, :], psf2[:FEAT, :B], AF.Identity,
                         bias=fb2_t[:])
    nc.vector.memset(featsT[FEAT:FEAT + 1, :], 1.0)

    # ---- head gate (softmax over heads, normalization folded) -------------
    psgl = prep.tile([128, 32], F32, tag="pre1")
    nc.tensor.matmul(psgl[:B, :B], featsT[:], gwt_t[:],
                     start=True, stop=True)
    gateb = cp.tile([32, 32], F32)          # gate[b, h] in [0:8, 0:8]
    nc.vector.memset(gateb[:], 0.0)
    nc.scalar.activation(gateb[:B, :B], psgl[:B, :B], AF.Exp)
    sums = cp.tile([B, 1], F32)
    nc.vector.tensor_reduce(sums[:], gateb[:B, :B], AX.X, ALU.add)
    recip = cp.tile([B, 1], F32)
    nc.vector.reciprocal(recip[:], sums[:])
    nc.vector.tensor_scalar_mul(gateb[:B, :B], gateb[:B, :B], recip[:])
    gatebT = cp.tile([32, 32], F32)         # gate[h, b] in [0:8, 0:8]
    nc.vector.transpose(gatebT[:], gateb[:])
    gatebT_bf = cp.tile([32, 32], BF16)
    nc.vector.tensor_copy(gatebT_bf[:], gatebT[:])
    # gate column per (pb, h): gcols[pb, h] = gate[h, pb % 8]
    psgc = prep.tile([128, 32], F32, tag="pre1")
    nc.tensor.matmul(psgc[:PB, :B], sel4_t[:], gatebT_bf[:B, :B],
                     start=True, stop=True)
    gcols = cp.tile([PB, B], F32)
    nc.vector.tensor_copy(gcols[:], psgc[:PB, :B])

    # ---- hinT (stationary for the gen_W1 matmuls) -------------------------
    for p in range(NP):
        nc.vector.tensor_copy(hinT[:FEAT, p * B:(p + 1) * B],
                              featsT[:FEAT, :])
    nc.sync.dma_start(hinT[FEAT:HIN, :], emb[:])
    nc.vector.memset(hinT[HIN:HIN + 1, :], 1.0)

    # ---- hinF (stationary of the att matmuls): feats x SC_F, onehot -------
    for p in range(NP):
        nc.scalar.mul(hinF[:FEAT, p * B:(p + 1) * B], featsT[:FEAT, :],
                      float(sc_f))
    nc.sync.dma_start(hinF[FEAT:KA, :], oh16[:])

    # ---- per-head gen stationary operand ----------------------------------
    if not GEN_COMB:
        tmp32 = cp.tile([PB, GH], F32, tag="tmpT")
    for h in range(H):
        psh = prep.tile([128, 32], F32, tag="preh")
        nc.tensor.matmul(psh[:PB, :GH], hinT[:], g1_t[:, h * GH:(h + 1) * GH],
                         start=True, stop=True)
        hmid = cp.tile([PB, GH], F32, tag="hmid")
        nc.scalar.activation(hmid[:], psh[:PB, :GH], AF.Relu)
        nc.vector.tensor_scalar_mul(hmid[:], hmid[:], gcols[:, h:h + 1])
        if GEN_COMB:
            hmid_bf = cp.tile([PB, GH], BF16, name="hmid_bf", tag="hmid_bf")
            nc.scalar.copy(hmid_bf[:], hmid[:])
            # lgen2 block-diag: out[32g+j, 32g'+pb] = d(g,g')*hmid[pb, j]
            psLG = prep.tile([128, 128], F32, tag="psLG")
            for g in range(4):
                nc.tensor.matmul(psLG[32 * g:32 * (g + 1), :], hmid_bf[:],
                                 i4_t[:, g * 128:(g + 1) * 128],
                                 start=True, stop=True,
                                 tile_position=(0, 32 * g))
            nc.vector.tensor_copy(lgen2[:, h * 128:(h + 1) * 128], psLG[:])
            nc.tensor.matmul(psh[GH:GH + 1, :PB], gatebT_bf[:B, h:h + 1],
                             sel4_t[:], start=True, stop=True,
                             tile_position=(0, 32))
            nc.scalar.mul(lgenb[:, h * PB:(h + 1) * PB],
                          psh[GH:GH + 1, :PB], float(SC_W))
            continue
        nc.vector.transpose(tmp32[:], hmid[:])
        nc.vector.tensor_copy(lgen[:GH, h * PB:(h + 1) * PB], tmp32[:])
        nc.tensor.matmul(psh[GH:GH + 1, :PB], gatebT_bf[:B, h:h + 1],
                         sel4_t[:], start=True, stop=True,
                         tile_position=(0, 32))
        if GEN_MIXED:
            nc.scalar.mul(lgenb[:, h * PB:(h + 1) * PB],
                          psh[GH:GH + 1, :PB], float(SC_W))
        else:
            nc.vector.tensor_copy(lgen[GH:GH + 1, h * PB:(h + 1) * PB],
                                  psh[GH:GH + 1, :PB])

    # ---- main streamed loop: whole-head DMAs, supertile compute -----------
    ts = out.shape[1]
    acc = accp.tile([128, ts // 4], F32, tag="acc")
    if OUT_BF16:
        accout = accp.tile([128, ts // 4], BF16, name="accout", tag="accb")
    else:
        accout = acc
    if ABLATE == "dma":
        nc.vector.memset(accout[:], 0.0)
    if RINGS == "3way":
        att_eng = [nc.sync] * 6 + [nc.scalar] * 2
        gen_eng = [nc.scalar] * 4 + [nc.gpsimd] * 4
        out_eng = nc.sync
    elif RINGS == "1way":
        att_eng = [nc.sync] * H
        gen_eng = [nc.sync] * H
        out_eng = nc.sync
    elif RINGS == "2bal":
        att_eng = [nc.sync] * H
        gen_eng = [nc.scalar] * H
        out_eng = nc.scalar
    elif RINGS == "2alt":
        att_eng = [nc.sync if h % 2 == 0 else nc.scalar for h in range(H)]
        gen_eng = [nc.scalar if h % 2 == 0 else nc.sync for h in range(H)]
        out_eng = nc.scalar
    else:
        att_eng = [nc.sync] * H
        gen_eng = [nc.scalar] * H
        out_eng = nc.sync
    first = {}
    for h in range(H):
        if ABLATE != "compute" or h == 0:
            att_t = sp.tile([KA, ts], att_dt, tag="att")
            att_eng[h].dma_start(att_t[:], att_in[h])
            if GEN_COMB:
                gen_t = sp.tile([4 * GH, ts // 4], F8, tag="gen")
                gen_eng[h].dma_start(gen_t[:], gen_in[h])
                genb_t = sp.tile([1, ts], BF16, tag="genb")
                gen_eng[h].dma_start(genb_t[:], genb_in[h])
            elif GEN_MIXED:
                gen_t = sp.tile([GH, ts], F8, tag="gen")
                gen_eng[h].dma_start(gen_t[:], gen_in[h])
                genb_t = sp.tile([1, ts], BF16, tag="genb")
                gen_eng[h].dma_start(genb_t[:], genb_in[h])
            else:
                gen_t = sp.tile([KG, ts], BF16, tag="gen")
                gen_eng[h].dma_start(gen_t[:], gen_in[h])
                genb_t = None
            first.setdefault("att", att_t)
            first.setdefault("gen", gen_t)
            first.setdefault("genb", genb_t)
        else:
            att_t, gen_t, genb_t = first["att"], first["gen"], first["genb"]
        if ABLATE == "dma":
            continue
        for s in range(n_sup + 1):
            ncols = SUP if s < n_sup else NSUB
            ns = ncols // 4
            c0 = s * SUP
            psA = pp.tile([128, NSUB], F32, tag="psA")
            psG = pp.tile([128, NSUB], F32, tag="psG")
            for g in range(4):
                nc.tensor.matmul(psA[32 * g:32 * (g + 1), :ns], hinF[:],
                                 att_t[:, c0 + g * ns:c0 + (g + 1) * ns],
                                 start=True, stop=True,
                                 tile_position=(0, 32 * g))
            if GEN_COMB:
                q0 = s * NSUB
                nc.tensor.matmul(psG[:, :ns],
                                 lgen2[:, h * 128:(h + 1) * 128],
                                 gen_t[:, q0:q0 + ns],
                                 start=True, stop=False)
                for g in range(4):
                    nc.tensor.matmul(psG[32 * g:32 * (g + 1), :ns],
                                     lgenb[:, h * PB:(h + 1) * PB],
                                     genb_t[:, c0 + g * ns:c0 + (g + 1) * ns],
                                     start=False, stop=True,
                                     tile_position=(0, 32 * g))
            elif GEN_MIXED:
                for g in range(4):
                    nc.tensor.matmul(psG[32 * g:32 * (g + 1), :ns],
                                     lgen[:GH, h * PB:(h + 1) * PB],
                                     gen_t[:, c0 + g * ns:c0 + (g + 1) * ns],
                                     start=True, stop=False,
                                     tile_position=(0, 32 * g))
                for g in range(4):
                    nc.tensor.matmul(psG[32 * g:32 * (g + 1), :ns],
                                     lgenb[:, h * PB:(h + 1) * PB],
                                     genb_t[:, c0 + g * ns:c0 + (g + 1) * ns],
                                     start=False, stop=True,
                                     tile_position=(0, 32 * g))
            else:
                for g in range(4):
                    nc.tensor.matmul(psG[32 * g:32 * (g + 1), :ns],
                                     lgen[:, h * PB:(h + 1) * PB],
                                     gen_t[:, c0 + g * ns:c0 + (g + 1) * ns],
                                     start=True, stop=True,
                                     tile_position=(0, 32 * g))
            imp = ev.tile([128, NSUB], F32, tag="imp")
            nc.scalar.activation(imp[:, :ns], psA[:, :ns], AF.Sigmoid,
                                 scale=float(inv_scale))
            a0 = s * NSUB
            gsc = 1.0 / SC_W if (GEN_MIXED or GEN_COMB) else 1.0
            if h == 0:
                nc.vector.scalar_tensor_tensor(acc[:, a0:a0 + ns],
                                               psG[:, :ns], float(gsc),
                                               imp[:, :ns],
                                               ALU.mult, ALU.mult)
            else:
                tmp = ev.tile([128, NSUB], F32, tag="tmp")
                nc.vector.scalar_tensor_tensor(tmp[:, :ns],
                                               psG[:, :ns], float(gsc),
                                               imp[:, :ns],
                                               ALU.mult, ALU.mult)
                dst = accout if (OUT_BF16 and h == H - 1) else acc
                nc.vector.tensor_add(dst[:, a0:a0 + ns], acc[:, a0:a0 + ns],
                                     tmp[:, :ns])
    for s in range(n_sup + 1):
        ncols = SUP if s < n_sup else NSUB
        ns = ncols // 4
        out_eng.dma_start(
            out[:, s * SUP:s * SUP + ncols].rearrange("p (g c) -> g p c", g=4),
            accout[:, s * NSUB:s * NSUB + ns])
    return nc


_NC_CACHE = None


def _get_nc():
    global _NC_CACHE
    if _NC_CACHE is None:
        _NC_CACHE = _build_bass()
    return _NC_CACHE


# ---------------------------------------------------------------------------
# Host wrapper
# ---------------------------------------------------------------------------
LAST_RESULTS = None  # BassKernelResults of the last run (for profiling)
LAST_IN_MAPS = None  # per-core input maps of the last run (for benchmarking)


def kernel(x, fe_W1, fe_b1, fe_W2, fe_b2, embeds,
           gen_W1, gen_b1, gen_W2, gen_b2, att_W, att_b,
           gate_W, gate_b):
    f32 = np.float32
    np_att = mybir.dt.np(F8 if ATT_FP8 else BF16)
    np_bf = mybir.dt.np(BF16)
    x = np.asarray(x, f32)
    fe_W1 = np.asarray(fe_W1, f32)
    fe_b1 = np.asarray(fe_b1, f32)
    fe_W2 = np.asarray(fe_W2, f32)
    fe_b2 = np.asarray(fe_b2, f32)
    embeds = np.asarray(embeds, f32)
    gen_W1 = np.asarray(gen_W1, f32)
    gen_b1 = np.asarray(gen_b1, f32)
    gen_W2 = np.asarray(gen_W2, f32)
    gen_b2 = np.asarray(gen_b2, f32)
    att_W = np.asarray(att_W, f32)
    att_b = np.asarray(att_b, f32)
    gate_W = np.asarray(gate_W, f32)
    gate_b = np.asarray(gate_b, f32)

    sc_w = SC_W if ATT_FP8 else 1.0
    # --- att stream: [H, 68, T_pad] (feats rows + rank-4 embeds/bias fold) --
    tpad = NCORES * TS
    att_all = np.zeros((H, KA, tpad), np_att)
    att_all[:, :FEAT, :T] = (att_W[:, :, :FEAT].transpose(0, 2, 1)
                             * sc_w).astype(np_att)
    F = np.einsum('pj,htj->hpt', embeds, att_W[:, :, FEAT:]) \
        + att_b[:, None, :]
    att_all[:, FEAT:, :T] = (F * sc_w).astype(np_att)
    # --- gen stream ---------------------------------------------------------
    if GEN_COMB:
        G8 = np.zeros((H, GH, tpad), np.float32)
        G8[:, :, :T] = gen_W2.transpose(0, 2, 1) * SC_W
        gen_all = G8  # regrouped per-core below
        genb_all = np.zeros((H, 1, tpad), np_bf)
        genb_all[:, 0, :T] = gen_b2.astype(np_bf)
    elif GEN_MIXED:
        np_f8 = mybir.dt.np(F8)
        gen_all = np.zeros((H, GH, tpad), np_f8)
        gen_all[:, :, :T] = (gen_W2.transpose(0, 2, 1) * SC_W).astype(np_f8)
        genb_all = np.zeros((H, 1, tpad), np_bf)
        genb_all[:, 0, :T] = gen_b2.astype(np_bf)
    else:
        gen_all = np.zeros((H, KG, tpad), np_bf)
        gen_all[:, :GH, :T] = gen_W2.transpose(0, 2, 1).astype(np_bf)
        gen_all[:, GH, :T] = gen_b2.astype(np_bf)
        genb_all = None

    # --- small shared operands ---
    xt = np.zeros((KFE, B), np_bf)
    xt[:784] = x.T.astype(np_bf)
    fe1t = np.zeros((KFE, 128), np_bf)
    fe1t[:784] = fe_W1.T.astype(np_bf)
    fb1 = np.ascontiguousarray(fe_b1[:, None])
    fw2t = np.ascontiguousarray(fe_W2.T.astype(np_bf))
    fb2 = np.ascontiguousarray(fe_b2[:, None])
    gwt = np.concatenate([gate_W.T, gate_b[None, :]], axis=0).astype(np_bf)
    emb = np.repeat(embeds.T[:, :, None], B, axis=2).reshape(EMB, PB)
    sel4 = np.tile(np.eye(B, dtype=f32), NP)
    g1in = np.concatenate([gen_W1.transpose(0, 2, 1), gen_b1[:, None, :]],
                          axis=1)                      # [H, 97, 32]
    g1in = g1in.transpose(1, 0, 2).reshape(HIN + 1, H * GH)
    oh = np.repeat(np.eye(NP, dtype=f32), B, axis=1) * (SC_OH if ATT_FP8
                                                        else 1.0)
    # selector constant for the combined-gen stationary build
    i4sel_a = np.zeros((PB, 4 * 128), f32)
    for g in range(4):
        for pb in range(PB):
            i4sel_a[pb, g * 128 + 32 * g + pb] = 1.0

    shared = {
        "xt": xt, "fe1t": fe1t, "fb1": fb1, "fw2t": fw2t, "fb2": fb2,
        "gwt": np.ascontiguousarray(gwt),
        "emb": np.ascontiguousarray(emb.astype(np_bf)),
        "sel4": np.ascontiguousarray(sel4.astype(np_bf)),
        "g1in": np.ascontiguousarray(g1in.astype(np_bf)),
        "oh16": np.ascontiguousarray(oh.astype(np_att)),
    }
    if GEN_COMB:
        shared["i4sel"] = np.ascontiguousarray(i4sel_a.astype(np_bf))

    def _regroup(a):
        # [H, k, TS] -> [H, 4k, TS//4]: column group g of each supertile
        # becomes contraction rows [g*k, (g+1)*k)
        k = a.shape[1]
        o = np.zeros((H, 4 * k, TS // 4), a.dtype)
        for s in range(7):
            c0 = s * SUP
            ns = (SUP if s < 6 else NSUB) // 4
            for g in range(4):
                o[:, g * k:(g + 1) * k, s * NSUB:s * NSUB + ns] = \
                    a[:, :, c0 + g * ns:c0 + (g + 1) * ns]
        return o

    in_maps = []
    for c in range(NCORES):
        sl = slice(c * TS, (c + 1) * TS)
        m = dict(shared)
        m["att_in"] = np.ascontiguousarray(att_all[:, :, sl])
        if GEN_COMB:
            np_f8 = mybir.dt.np(F8)
            m["gen_in"] = np.ascontiguousarray(
                _regroup(gen_all[:, :, sl]).astype(np_f8))
            m["genb_in"] = np.ascontiguousarray(genb_all[:, :, sl])
        else:
            m["gen_in"] = np.ascontiguousarray(gen_all[:, :, sl])
            if genb_all is not None:
                m["genb_in"] = np.ascontiguousarray(genb_all[:, :, sl])
        in_maps.append(m)

    nc = _get_nc()
    res = run_bass_kernel_spmd(nc, in_maps, core_ids=list(range(NCORES)))
    global LAST_RESULTS, LAST_IN_MAPS
    LAST_RESULTS = res
    LAST_IN_MAPS = in_maps

    full = np.concatenate(
        [np.asarray(res.results[c]["out"], np.float32) for c in range(NCORES)],
        axis=1)[:, :T]                                # [32, T], row = p*8+b
    return np.ascontiguousarray(
        full.reshape(NP, B, T).transpose(1, 0, 2).reshape(B, NP * T))


# ---------------------------------------------------------------------------
# Timing harness (test-only): device-resident inputs, repeated execution.
# ---------------------------------------------------------------------------
def benchmark_last(in_maps, iters=8, nc=None):
    import time

    import jax
    from concourse import bass2jax as b2j
    from concourse import mybir as _mybir

    if nc is None:
        nc = _get_nc()
    b2j.install_neuronx_cc_hook()

    partition_name = (nc.partition_id_tensor.name
                      if nc.partition_id_tensor else None)
    in_names, out_names, out_avals, zero_outs = [], [], [], []
    for alloc in nc.m.functions[0].allocations:
        if not isinstance(alloc, _mybir.MemoryLocationSet):
            continue
        name = alloc.memorylocations[0].name
        if alloc.kind == "ExternalInput":
            if name != partition_name:
                in_names.append(name)
        elif alloc.kind == "ExternalOutput":
            shape = tuple(alloc.tensor_shape)
            dtype = _mybir.dt.np(alloc.dtype)
            out_names.append(name)
            out_avals.append(jax.core.ShapedArray(shape, dtype))
            zero_outs.append(np.zeros(shape, dtype))
    n_params = len(in_names)
    n_outs = len(out_avals)
    in_names_all = in_names + out_names
    if partition_name is not None:
        in_names_all.append(partition_name)

    def _body(*args):
        operands = list(args)
        if partition_name is not None:
            operands.append(b2j.partition_id_tensor())
        return tuple(b2j._bass_exec_p.bind(
            *operands,
            out_avals=tuple(out_avals),
            in_names=tuple(in_names_all),
            out_names=tuple(out_names),
            lowering_input_output_aliases=(),
            sim_require_finite=True,
            sim_require_nnan=True,
            nc=nc,
        ))

    donate = tuple(range(n_params, n_params + n_outs))
    devices = jax.devices()[:NCORES]
    mesh = b2j.Mesh(np.asarray(devices), ("core",))
    sharded = jax.jit(
        b2j.shard_map(_body, mesh=mesh,
                      in_specs=(b2j.PartitionSpec("core"),) * (n_params + n_outs),
                      out_specs=(b2j.PartitionSpec("core"),) * n_outs,
                      check_rep=False),
        donate_argnums=donate, keep_unused=True)

    concat_in = [
        np.concatenate([np.asarray(in_maps[c][nm]) for c in range(NCORES)],
                       axis=0)
        for nm in in_names
    ]
    sharding = jax.sharding.NamedSharding(mesh, b2j.PartitionSpec("core"))
    dev_in = [jax.device_put(a, sharding) for a in concat_in]

    def _zeros():
        return [jax.device_put(
            np.zeros((NCORES * z.shape[0], *z.shape[1:]), z.dtype), sharding)
            for z in zero_outs]

    # warmup (compile + load)
    outs = sharded(*dev_in, *_zeros())
    jax.block_until_ready(outs)
    times = []
    for _ in range(iters):
        zs = _zeros()
        jax.block_until_ready(zs)
        t0 = time.perf_counter()
        outs = sharded(*dev_in, *zs)
        jax.block_until_ready(outs)
        times.append(time.perf_counter() - t0)
    return min(times), times

